# revision 1
# baseline (speedup 1.0000x reference)
"""Winding-number field (differentiable voxelizer) on 8 Trainium2 NeuronCores.

Strategy (data-parallel over query points, per sharding hint):
  Host (cheap, O(V+F+near-pairs)):
    - areaic normals `na` + per-vertex `d = v.na` replicated bit-exactly from
      the reference's fp32 jax ops (CPU).
    - near-pair correction: for all (point, vert) pairs with r < RCUT, swap the
      predicted device term for the exact reference term (fp64).
  Device (8 cores, the P*V = 2.7e8-pair field, fp32):
    - mm1 (TensorE): r2[v,p] = vv + pp - 2 v.p  via K=5 fp32 matmul
    - ScalarE:       r = sqrt(r2 + 1e-12)
    - VectorE:       denom = (r2 + B_REG) * r        (fused scalar_tensor_tensor)
    - VectorE:       s = reciprocal_approx_fast(denom)
    - mm2 (TensorE): A_c[p] = sum_v s*na_c, B[p] = sum_v s*d  (PSUM accumulation)
  Host combine: wf = (B - p.A) / 4pi  + near-pair correction.

The device regularizer B_REG bounds the sensitivity of near-pair terms to the
fp32 cancellation in the Gram-form r2, which makes the device value of every
near pair predictable on the host, so the correction is exact.
"""

import os
import sys

import numpy as np

for _p in ("/opt/trn_rl_repo", "/root/.axon_site/_ro/trn_rl_repo"):
    if _p not in sys.path and os.path.isdir(_p):
        sys.path.insert(0, _p)

from contextlib import ExitStack

import concourse.bass as bass  # noqa: E402
import concourse.tile as tile  # noqa: E402
from concourse import bacc, mybir  # noqa: E402
from concourse.bass_utils import run_bass_kernel_spmd  # noqa: E402

EPS = 1e-8          # reference epsilon in 1/(r^3 + EPS)
B_REG = 1e-4        # device regularizer: denom = (r2 + B_REG) * r
SQRT_BIAS = 1e-12   # guards r2 == 0 exactly
RCUT = 0.3          # host-corrected pair radius
FOUR_PI = 4.0 * np.pi

# v2 path: s = Exp(-1.5 * Ln(r2 + B_REG)) = (r2+b)^-1.5 — two ScalarE ops in
# one activation table set (natural_log_exp_and_others), VectorE left idle.
USE_EXPLN = True
# (measured: this backend charges ~35-80us per *instruction*, so fewer/larger
# ops win on both axes — a hybrid scalar/vector split would add instructions)
HYBRID = False

N_CORES = 8
V = 8192
P = 32768
PC = P // N_CORES         # 4096 points per core
PB = 512                  # point block (one fp32 matmul moving limit / PSUM bank)
VT = 128                  # vert tile (partition dim)
VG = 2                    # vert tiles per elementwise group (FD = VG*PB = 1024)
F32 = mybir.dt.float32

_NC_CACHE = {}


class _OneSetBacc(bacc.Bacc):
    """Bacc whose activation-table pass only sees `natural_log_exp_and_others`.

    The default pass maps Ln -> `natural_log` and Exp -> `exp_and_others`,
    inserting an ACT_TABLE_LOAD (~2.7us) before nearly every activation when
    Ln/Exp alternate. One set contains ln, exp, and copy — restricting the
    candidate list makes the pass hoist a single load."""

    def insert_act_table_loads(self):
        import bass_rust as _bass_rust
        from concourse.hw_specs import get_activation_tables

        has_activation = any(
            isinstance(i, mybir.InstActivation)
            for b in self.main_func.blocks
            for i in b.instructions
        )
        if not has_activation:
            return
        # Keep list positions (act_func_set_id is positional) but blank out
        # every other set so the pass can only pick the ones we use.
        keep = {"natural_log_exp_and_others"} | ({"sqrt_and_others"} if HYBRID else set())
        tables = [(k, v if k in keep else set())
                  for k, v in get_activation_tables(self.m.arch).items()]
        assert any(v for _, v in tables), "required activation sets missing"
        _bass_rust.insert_act_table_loads(self, tables)


def _build_nc(reps=1, work_bufs=3, po_bufs=2):
    """Build the SPMD Bass module (same program for all 8 cores).

    reps>1 repeats the whole computation (identical output) — used only for
    marginal-time measurement: device_time ≈ (wall(N) - wall(1)) / (N-1)."""
    nc = (_OneSetBacc if USE_EXPLN else bacc.Bacc)(
        "TRN2", target_bir_lowering=False, debug=False)

    vl5_d = nc.declare_dram_parameter("vl5", [5, V], F32, isOutput=False)
    pts5_d = nc.declare_dram_parameter("pts5", [5, PC], F32, isOutput=False)
    n4_d = nc.declare_dram_parameter("n4", [VT, (V // VT) * 4], F32, isOutput=False)
    out4_d = nc.declare_dram_parameter("out4", [4, PC], F32, isOutput=True)

    n_pb = PC // PB                 # 8 point blocks
    n_vt = V // VT                  # 64 vert tiles
    n_vg = n_vt // VG               # 32 groups
    FD = VG * PB                    # 1024 elementwise free dim

    with ExitStack() as ctx:
        tc = ctx.enter_context(tile.TileContext(nc))
        consts = ctx.enter_context(tc.tile_pool(name="consts", bufs=1))
        work = ctx.enter_context(tc.tile_pool(name="work", bufs=work_bufs))
        psum_r2 = ctx.enter_context(tc.tile_pool(name="psum_r2", bufs=3, space="PSUM"))
        psum_out = ctx.enter_context(
            tc.tile_pool(name="psum_out", bufs=po_bufs, space="PSUM"))

        vl5 = consts.tile([5, V], F32)
        pts5 = consts.tile([5, PC], F32)
        n4 = consts.tile([VT, (V // VT) * 4], F32)
        outsb = consts.tile([4, PC], F32)
        sqrt_bias = consts.tile([VT, 1], F32)
        nc.vector.memset(sqrt_bias[:], B_REG if USE_EXPLN else SQRT_BIAS)
        nc.sync.dma_start(out=vl5[:], in_=vl5_d.ap())
        nc.sync.dma_start(out=pts5[:], in_=pts5_d.ap())
        nc.sync.dma_start(out=n4[:], in_=n4_d.ap())

        # Per-instruction overhead dominates on this backend and per-engine
        # streams run concurrently, so keep the PE stream deeply buffered
        # (psum bufs=3) and batch ScalarE ops: Ln per 2-vt PSUM chunk
        # (FD 1024), one Exp per u-group of 4 chunks (FD 4096). Shrinking
        # PSUM buffering to enlarge Ln chunks measured *slower* (PE stalls).
        chunks = [VG] * (n_vt // VG)
        ugroups = [chunks[i:i + 4] for i in range(0, len(chunks), 4)]
        for rep in range(reps):
          for pb in range(n_pb):
            acc = psum_out.tile([4, PB], F32)
            vt_base = 0
            for ug in ugroups:
                nvt_g = sum(ug)
                u = work.tile([VT, nvt_g * PB], F32, tag="u")
                off = 0
                g_vt0 = vt_base
                for ch in ug:
                    r2 = psum_r2.tile([VT, ch * PB], F32, tag="r2")
                    for i in range(ch):
                        vt = vt_base + i
                        nc.tensor.matmul(
                            r2[:, i * PB:(i + 1) * PB],
                            vl5[:, vt * VT:(vt + 1) * VT],
                            pts5[:, pb * PB:(pb + 1) * PB],
                            start=True,
                            stop=True,
                        )
                    # u = Ln(r2 + B_REG)   (s = Exp(-1.5*u) = (r2+b)^-1.5)
                    nc.scalar.activation(u[:, off:off + ch * PB], r2[:],
                                         mybir.ActivationFunctionType.Ln,
                                         bias=sqrt_bias[:])
                    off += ch * PB
                    vt_base += ch
                s = work.tile([VT, nvt_g * PB], F32, tag="s")
                nc.scalar.activation(s[:], u[:],
                                     mybir.ActivationFunctionType.Exp,
                                     scale=-1.5)
                for l in range(nvt_g):
                    vt = g_vt0 + l
                    nc.tensor.matmul(
                        acc[:],
                        n4[:, vt * 4:(vt + 1) * 4],
                        s[:, l * PB:(l + 1) * PB],
                        start=(vt == 0),
                        stop=(vt == n_vt - 1),
                    )
            nc.vector.tensor_copy(outsb[:, pb * PB:(pb + 1) * PB], acc[:])
        nc.sync.dma_start(out=out4_d.ap(), in_=outsb[:])
    nc.finalize()
    return nc


# ------------------------- host-side numerics --------------------------------
def _preprocess_mesh(verts, faces):
    """Bit-exact replica of the reference's areaic normals: jax fp32 on CPU."""
    import jax
    import jax.numpy as jnp

    with jax.default_device(jax.devices("cpu")[0]):
        v = jnp.asarray(verts, jnp.float32)
        f = jnp.asarray(np.asarray(faces).astype(np.int32))
        fv = v[f]
        A = fv[:, 1] - fv[:, 0]
        Bv = fv[:, 2] - fv[:, 1]
        C = fv[:, 0] - fv[:, 2]

        def corner_angle(u, w):
            c = -jnp.sum(u * w, axis=1) / (
                EPS + jnp.linalg.norm(u, axis=1) * jnp.linalg.norm(w, axis=1))
            return jnp.arccos(jnp.clip(c, -1.0, 1.0))

        angles = jnp.stack(
            [corner_angle(C, A), corner_angle(A, Bv), corner_angle(Bv, C)], axis=1)
        s2 = jnp.sin(2.0 * angles)
        w = s2 / (jnp.sum(s2, axis=-1, keepdims=True) + EPS)
        w = (w[:, [2, 0, 1]] + w[:, [1, 2, 0]]) / 2.0

        fn = jnp.cross(A, Bv)
        areas = 0.5 * jnp.linalg.norm(fn, axis=1)

        nv = v.shape[0]
        idx = f.reshape(-1)
        dual_v = jax.ops.segment_sum((w * areas[:, None]).reshape(-1), idx,
                                     num_segments=nv)
        vn = jax.ops.segment_sum(jnp.repeat(fn, 3, axis=0), idx, num_segments=nv)
        vn = vn / (jnp.linalg.norm(vn, axis=1, keepdims=True) + EPS)
        na = np.asarray(vn * dual_v[:, None])
    d = np.sum(na.astype(np.float64) * np.asarray(verts, np.float64), axis=1)
    return na, d.astype(np.float32)


def _near_pairs(points, verts, rcut):
    """(point, vert) pairs with |p-v| < rcut via grid hashing (pure numpy)."""
    from collections import defaultdict

    pts = points.astype(np.float64)
    vts = verts.astype(np.float64)
    vcell = np.floor(vts / rcut).astype(np.int64)
    vmap = defaultdict(list)
    for j, c in enumerate(map(tuple, vcell)):
        vmap[c].append(j)
    vmap = {k: np.asarray(vs) for k, vs in vmap.items()}
    pcell = np.floor(pts / rcut).astype(np.int64)
    order = np.lexsort((pcell[:, 2], pcell[:, 1], pcell[:, 0]))
    pc_sorted = pcell[order]
    bounds = np.nonzero(np.any(np.diff(pc_sorted, axis=0) != 0, axis=1))[0] + 1
    starts = np.concatenate([[0], bounds])
    ends = np.concatenate([bounds, [len(order)]])
    ip_list, iv_list = [], []
    for s0, e0 in zip(starts, ends):
        pidx = order[s0:e0]
        c = pc_sorted[s0]
        cand = [vmap[k] for k in
                ((c[0] + dx, c[1] + dy, c[2] + dz)
                 for dx in (-1, 0, 1) for dy in (-1, 0, 1) for dz in (-1, 0, 1))
                if k in vmap]
        if not cand:
            continue
        cand = np.concatenate(cand)
        diff = vts[None, cand, :] - pts[pidx, None, :]
        r2 = np.sum(diff * diff, axis=2)
        ii, jj = np.nonzero(r2 < rcut * rcut)
        ip_list.append(pidx[ii])
        iv_list.append(cand[jj])
    if not ip_list:
        return np.zeros(0, np.int64), np.zeros(0, np.int64)
    return np.concatenate(ip_list), np.concatenate(iv_list)


def _host_correction(points32, verts32, na, pp32, vv32):
    """wf_corr[p] = sum_near [s_true - s_devpred] * (na_v . (v-p)) / 4pi."""
    ip, iv = _near_pairs(points32, verts32, RCUT)
    p = points32.astype(np.float64)[ip]
    v = verts32.astype(np.float64)[iv]
    diff = v - p
    r2t = np.sum(diff * diff, axis=1)
    s_true = 1.0 / (r2t ** 1.5 + EPS)
    # predicted device values (device rounding differs ~2e-7 abs in r2; the
    # B_REG floor makes that negligible relative to s_true)
    r2d = vv32.astype(np.float64)[iv] + pp32.astype(np.float64)[ip] \
        - 2.0 * np.sum(p * v, axis=1)
    r2d = np.maximum(r2d, 0.0)
    if USE_EXPLN:
        s_dev = (r2d + B_REG) ** -1.5
    else:
        s_dev = 1.0 / ((r2d + B_REG) * np.sqrt(r2d + SQRT_BIAS))
    g = np.sum(na.astype(np.float64)[iv] * diff, axis=1)
    corr = (s_true - s_dev) * g / FOUR_PI
    return np.bincount(ip, weights=corr, minlength=points32.shape[0])


# ------------------------------- entry point ---------------------------------
def _prepare(verts, points, faces):
    verts32 = np.ascontiguousarray(np.asarray(verts, np.float32))
    points32 = np.ascontiguousarray(np.asarray(points, np.float32))

    na, d = _preprocess_mesh(verts32, faces)

    vv32 = np.sum(verts32.astype(np.float64) ** 2, axis=1).astype(np.float32)
    pp32 = np.sum(points32.astype(np.float64) ** 2, axis=1).astype(np.float32)

    vl5 = np.empty((5, V), np.float32)
    vl5[0:3] = verts32.T
    vl5[3] = 1.0
    vl5[4] = vv32

    # n4: lhsT tiles for mm2 — n4[vp, vt*4+j] = [na | d][vt*128+vp, j]
    nmat = np.concatenate([na.astype(np.float32), d[:, None]], axis=1)  # (V,4)
    n4 = np.ascontiguousarray(
        nmat.reshape(V // VT, VT, 4).transpose(1, 0, 2).reshape(VT, (V // VT) * 4))

    in_maps = []
    for c in range(N_CORES):
        sl = slice(c * PC, (c + 1) * PC)
        pts5 = np.empty((5, PC), np.float32)
        pts5[0:3] = np.float32(-2.0) * points32[sl].T
        pts5[3] = pp32[sl]
        pts5[4] = 1.0
        in_maps.append({"vl5": vl5, "pts5": pts5, "n4": n4})
    return in_maps, verts32, points32, na, pp32, vv32


def _finish(core_outs, verts32, points32, na, pp32, vv32):
    """core_outs: list of (4, PC) arrays. Combine + near-pair correction."""
    wf = np.empty(P, np.float64)
    for c in range(N_CORES):
        sl = slice(c * PC, (c + 1) * PC)
        o = np.asarray(core_outs[c], np.float64)
        pd = points32[sl].astype(np.float64)
        wf[sl] = (o[3] - pd[:, 0] * o[0] - pd[:, 1] * o[1] - pd[:, 2] * o[2]) / FOUR_PI
    wf += _host_correction(points32, verts32, na, pp32, vv32)
    return wf.astype(np.float32)


def kernel(verts, points, faces):
    import time

    in_maps, verts32, points32, na, pp32, vv32 = _prepare(verts, points, faces)
    last_err = None
    for attempt in range(3):
        try:
            if "nc" not in _NC_CACHE:
                _NC_CACHE["nc"] = _build_nc()
            res = run_bass_kernel_spmd(_NC_CACHE["nc"], in_maps,
                                       list(range(N_CORES)))
            core_outs = [np.asarray(res.results[c]["out4"])
                         for c in range(N_CORES)]
            break
        except Exception as e:  # transient axon/NRT faults: rebuild + retry
            last_err = e
            _NC_CACHE.clear()
            time.sleep(5 * (attempt + 1))
    else:
        raise last_err
    return _finish(core_outs, verts32, points32, na, pp32, vv32)



# revision 2
# speedup vs baseline: 9.6583x; 9.6583x over previous
"""Winding-number field (differentiable voxelizer) on 8 Trainium2 NeuronCores.

Looped variant: the measured per-run cost on this backend is ~60-90us per
STATIC instruction (program size), while dynamic instructions are nearly free
(probe: 512 matmuls in a For_i loop with an 8-instruction body cost the same
wall time as 64 unrolled matmuls). So the unrolled baseline (1352 static
instructions, ~83ms) is rebuilt as hardware loops:

  for j in For_i(n_ug):        # 8 groups of 8 vert tiles
    wslot <- vl5[:, j*1024 +: 1024]   (dynamic-offset copy: matmul weights
    nslot <- n4[:, j*32 +: 32]         must have static addresses)
    for i in For_i(n_pb):      # 8 point blocks
      8x mm1 (r2 via Gram form) -> 4x Ln -> 1x Exp -> 8x mm2 (PSUM acc)
      outsb[:, i*512 +: 512] += acc    (VectorE, dynamic offset)

Static instructions ~= 60 instead of 1352; device math is unchanged
(s = Exp(-1.5*Ln(r2 + B_REG)), same tile shapes), so the host-side
near-pair correction remains exact.

Strategy otherwise identical to the unrolled baseline (see git history):
data-parallel over query points, host computes areaic normals bit-exactly
and corrects all pairs with r < RCUT in fp64.
"""

import os
import sys

import numpy as np

for _p in ("/opt/trn_rl_repo", "/root/.axon_site/_ro/trn_rl_repo"):
    if _p not in sys.path and os.path.isdir(_p):
        sys.path.insert(0, _p)

from contextlib import ExitStack

import concourse.bass as bass  # noqa: E402
import concourse.tile as tile  # noqa: E402
from concourse import bacc, mybir  # noqa: E402
from concourse.bass import ds  # noqa: E402
from concourse.bass_utils import run_bass_kernel_spmd  # noqa: E402

EPS = 1e-8          # reference epsilon in 1/(r^3 + EPS)
B_REG = 1e-4        # device regularizer: s = (r2 + B_REG)^-1.5
RCUT = 0.3          # host-corrected pair radius
FOUR_PI = 4.0 * np.pi

N_CORES = 8
V = 8192
P = 32768
PC = P // N_CORES         # 4096 points per core
PB = 512                  # point block (one fp32 matmul moving limit / PSUM bank)
VT = 128                  # vert tile (partition dim)
VG = 2                    # vert tiles per Ln chunk (FD = VG*PB = 1024)
UG = 8                    # vert tiles per loop iteration (FD_ug = UG*PB = 4096)
F32 = mybir.dt.float32

_NC_CACHE = {}


class _OneSetBacc(bacc.Bacc):
    """Bacc whose activation-table pass only sees `natural_log_exp_and_others`
    (contains ln, exp, copy) so a single ACT_TABLE_LOAD is hoisted instead of
    one per Ln<->Exp alternation."""

    def insert_act_table_loads(self):
        import bass_rust as _bass_rust
        from concourse.hw_specs import get_activation_tables

        has_activation = any(
            isinstance(i, mybir.InstActivation)
            for b in self.main_func.blocks
            for i in b.instructions
        )
        if not has_activation:
            return
        keep = {"natural_log_exp_and_others"}
        tables = [(k, v if k in keep else set())
                  for k, v in get_activation_tables(self.m.arch).items()]
        assert any(v for _, v in tables), "required activation sets missing"
        _bass_rust.insert_act_table_loads(self, tables)


def _build_nc(reps=1):
    """Build the SPMD Bass module (same program for all 8 cores).

    reps>1 repeats the whole computation (identical output) — used only for
    marginal-time measurement: device_time ~= (wall(N) - wall(1)) / (N-1)."""
    nc = _OneSetBacc("TRN2", target_bir_lowering=False, debug=False)

    vl5_d = nc.declare_dram_parameter("vl5", [5, V], F32, isOutput=False)
    pts5_d = nc.declare_dram_parameter("pts5", [5, PC], F32, isOutput=False)
    n4_d = nc.declare_dram_parameter("n4", [VT, (V // VT) * 4], F32, isOutput=False)
    out4_d = nc.declare_dram_parameter("out4", [4, PC], F32, isOutput=True)

    n_pb = PC // PB                 # 8 point blocks
    n_vt = V // VT                  # 64 vert tiles
    n_ug = n_vt // UG               # 8 vert-tile groups (outer loop)
    n_ch = UG // VG                 # 4 Ln chunks per group

    with ExitStack() as ctx:
        tc = ctx.enter_context(tile.TileContext(nc))
        consts = ctx.enter_context(tc.tile_pool(name="consts", bufs=1))
        psum_r2 = ctx.enter_context(tc.tile_pool(name="psum_r2", bufs=3, space="PSUM"))
        psum_out = ctx.enter_context(tc.tile_pool(name="psum_out", bufs=1, space="PSUM"))

        vl5 = consts.tile([5, V], F32)
        pts5 = consts.tile([5, PC], F32)
        n4 = consts.tile([VT, (V // VT) * 4], F32)
        outsb = consts.tile([4, PC], F32)
        sqrt_bias = consts.tile([VT, 1], F32)
        wslot = consts.tile([5, UG * VT], F32)
        nslot = consts.tile([VT, UG * 4], F32)
        u = consts.tile([VT, UG * PB], F32)
        s = consts.tile([VT, UG * PB], F32)
        nc.vector.memset(sqrt_bias[:], B_REG)
        nc.vector.memset(outsb[:], 0.0)
        nc.sync.dma_start(out=vl5[:], in_=vl5_d.ap())
        nc.sync.dma_start(out=pts5[:], in_=pts5_d.ap())
        nc.sync.dma_start(out=n4[:], in_=n4_d.ap())

        for rep in range(reps):
            if rep > 0:
                # reps replicate the whole computation; reset the accumulator
                # so every rep produces the identical output.
                nc.vector.memset(outsb[:], 0.0)
            with tc.For_i(0, n_ug) as j:
                nc.vector.tensor_copy(wslot[:], vl5[:, ds(j * (UG * VT), UG * VT)])
                nc.vector.tensor_copy(nslot[:], n4[:, ds(j * (UG * 4), UG * 4)])
                with tc.For_i(0, n_pb) as i:
                    for ch in range(n_ch):
                        r2 = psum_r2.tile([VT, VG * PB], F32, tag="r2")
                        for t in range(VG):
                            k = ch * VG + t
                            nc.tensor.matmul(
                                r2[:, t * PB:(t + 1) * PB],
                                wslot[:, k * VT:(k + 1) * VT],
                                pts5[:, ds(i * PB, PB)],
                                start=True,
                                stop=True,
                            )
                        # u = Ln(r2 + B_REG)   (s = Exp(-1.5*u) = (r2+b)^-1.5)
                        nc.scalar.activation(u[:, ch * VG * PB:(ch + 1) * VG * PB],
                                             r2[:],
                                             mybir.ActivationFunctionType.Ln,
                                             bias=sqrt_bias[:])
                    nc.scalar.activation(s[:], u[:],
                                         mybir.ActivationFunctionType.Exp,
                                         scale=-1.5)
                    acc = psum_out.tile([4, PB], F32, tag="acc")
                    for k in range(UG):
                        nc.tensor.matmul(
                            acc[:],
                            nslot[:, k * 4:(k + 1) * 4],
                            s[:, k * PB:(k + 1) * PB],
                            start=(k == 0),
                            stop=(k == UG - 1),
                        )
                    nc.vector.tensor_add(outsb[:, ds(i * PB, PB)],
                                         outsb[:, ds(i * PB, PB)], acc[:])
        nc.sync.dma_start(out=out4_d.ap(), in_=outsb[:])
    nc.finalize()
    return nc


# ------------------------- host-side numerics --------------------------------
def _preprocess_mesh(verts, faces):
    """Bit-exact replica of the reference's areaic normals: jax fp32 on CPU."""
    import jax
    import jax.numpy as jnp

    with jax.default_device(jax.devices("cpu")[0]):
        v = jnp.asarray(verts, jnp.float32)
        f = jnp.asarray(np.asarray(faces).astype(np.int32))
        fv = v[f]
        A = fv[:, 1] - fv[:, 0]
        Bv = fv[:, 2] - fv[:, 1]
        C = fv[:, 0] - fv[:, 2]

        def corner_angle(u, w):
            c = -jnp.sum(u * w, axis=1) / (
                EPS + jnp.linalg.norm(u, axis=1) * jnp.linalg.norm(w, axis=1))
            return jnp.arccos(jnp.clip(c, -1.0, 1.0))

        angles = jnp.stack(
            [corner_angle(C, A), corner_angle(A, Bv), corner_angle(Bv, C)], axis=1)
        s2 = jnp.sin(2.0 * angles)
        w = s2 / (jnp.sum(s2, axis=-1, keepdims=True) + EPS)
        w = (w[:, [2, 0, 1]] + w[:, [1, 2, 0]]) / 2.0

        fn = jnp.cross(A, Bv)
        areas = 0.5 * jnp.linalg.norm(fn, axis=1)

        nv = v.shape[0]
        idx = f.reshape(-1)
        dual_v = jax.ops.segment_sum((w * areas[:, None]).reshape(-1), idx,
                                     num_segments=nv)
        vn = jax.ops.segment_sum(jnp.repeat(fn, 3, axis=0), idx, num_segments=nv)
        vn = vn / (jnp.linalg.norm(vn, axis=1, keepdims=True) + EPS)
        na = np.asarray(vn * dual_v[:, None])
    d = np.sum(na.astype(np.float64) * np.asarray(verts, np.float64), axis=1)
    return na, d.astype(np.float32)


def _near_pairs(points, verts, rcut):
    """(point, vert) pairs with |p-v| < rcut via grid hashing (pure numpy)."""
    from collections import defaultdict

    pts = points.astype(np.float64)
    vts = verts.astype(np.float64)
    vcell = np.floor(vts / rcut).astype(np.int64)
    vmap = defaultdict(list)
    for j, c in enumerate(map(tuple, vcell)):
        vmap[c].append(j)
    vmap = {k: np.asarray(vs) for k, vs in vmap.items()}
    pcell = np.floor(pts / rcut).astype(np.int64)
    order = np.lexsort((pcell[:, 2], pcell[:, 1], pcell[:, 0]))
    pc_sorted = pcell[order]
    bounds = np.nonzero(np.any(np.diff(pc_sorted, axis=0) != 0, axis=1))[0] + 1
    starts = np.concatenate([[0], bounds])
    ends = np.concatenate([bounds, [len(order)]])
    ip_list, iv_list = [], []
    for s0, e0 in zip(starts, ends):
        pidx = order[s0:e0]
        c = pc_sorted[s0]
        cand = [vmap[k] for k in
                ((c[0] + dx, c[1] + dy, c[2] + dz)
                 for dx in (-1, 0, 1) for dy in (-1, 0, 1) for dz in (-1, 0, 1))
                if k in vmap]
        if not cand:
            continue
        cand = np.concatenate(cand)
        diff = vts[None, cand, :] - pts[pidx, None, :]
        r2 = np.sum(diff * diff, axis=2)
        ii, jj = np.nonzero(r2 < rcut * rcut)
        ip_list.append(pidx[ii])
        iv_list.append(cand[jj])
    if not ip_list:
        return np.zeros(0, np.int64), np.zeros(0, np.int64)
    return np.concatenate(ip_list), np.concatenate(iv_list)


def _host_correction(points32, verts32, na, pp32, vv32):
    """wf_corr[p] = sum_near [s_true - s_devpred] * (na_v . (v-p)) / 4pi."""
    ip, iv = _near_pairs(points32, verts32, RCUT)
    p = points32.astype(np.float64)[ip]
    v = verts32.astype(np.float64)[iv]
    diff = v - p
    r2t = np.sum(diff * diff, axis=1)
    s_true = 1.0 / (r2t ** 1.5 + EPS)
    # predicted device values (device rounding differs ~2e-7 abs in r2; the
    # B_REG floor makes that negligible relative to s_true)
    r2d = vv32.astype(np.float64)[iv] + pp32.astype(np.float64)[ip] \
        - 2.0 * np.sum(p * v, axis=1)
    r2d = np.maximum(r2d, 0.0)
    s_dev = (r2d + B_REG) ** -1.5
    g = np.sum(na.astype(np.float64)[iv] * diff, axis=1)
    corr = (s_true - s_dev) * g / FOUR_PI
    return np.bincount(ip, weights=corr, minlength=points32.shape[0])


# ------------------------------- entry point ---------------------------------
def _prepare(verts, points, faces):
    verts32 = np.ascontiguousarray(np.asarray(verts, np.float32))
    points32 = np.ascontiguousarray(np.asarray(points, np.float32))

    na, d = _preprocess_mesh(verts32, faces)

    vv32 = np.sum(verts32.astype(np.float64) ** 2, axis=1).astype(np.float32)
    pp32 = np.sum(points32.astype(np.float64) ** 2, axis=1).astype(np.float32)

    vl5 = np.empty((5, V), np.float32)
    vl5[0:3] = verts32.T
    vl5[3] = 1.0
    vl5[4] = vv32

    # n4: lhsT tiles for mm2 — n4[vp, vt*4+j] = [na | d][vt*128+vp, j]
    nmat = np.concatenate([na.astype(np.float32), d[:, None]], axis=1)  # (V,4)
    n4 = np.ascontiguousarray(
        nmat.reshape(V // VT, VT, 4).transpose(1, 0, 2).reshape(VT, (V // VT) * 4))

    in_maps = []
    for c in range(N_CORES):
        sl = slice(c * PC, (c + 1) * PC)
        pts5 = np.empty((5, PC), np.float32)
        pts5[0:3] = np.float32(-2.0) * points32[sl].T
        pts5[3] = pp32[sl]
        pts5[4] = 1.0
        in_maps.append({"vl5": vl5, "pts5": pts5, "n4": n4})
    return in_maps, verts32, points32, na, pp32, vv32


def _finish(core_outs, verts32, points32, na, pp32, vv32):
    """core_outs: list of (4, PC) arrays. Combine + near-pair correction."""
    wf = np.empty(P, np.float64)
    for c in range(N_CORES):
        sl = slice(c * PC, (c + 1) * PC)
        o = np.asarray(core_outs[c], np.float64)
        pd = points32[sl].astype(np.float64)
        wf[sl] = (o[3] - pd[:, 0] * o[0] - pd[:, 1] * o[1] - pd[:, 2] * o[2]) / FOUR_PI
    wf += _host_correction(points32, verts32, na, pp32, vv32)
    return wf.astype(np.float32)


def kernel(verts, points, faces):
    import time

    in_maps, verts32, points32, na, pp32, vv32 = _prepare(verts, points, faces)
    last_err = None
    for attempt in range(3):
        try:
            if "nc" not in _NC_CACHE:
                _NC_CACHE["nc"] = _build_nc()
            res = run_bass_kernel_spmd(_NC_CACHE["nc"], in_maps,
                                       list(range(N_CORES)))
            core_outs = [np.asarray(res.results[c]["out4"])
                         for c in range(N_CORES)]
            break
        except Exception as e:  # transient axon/NRT faults: rebuild + retry
            last_err = e
            _NC_CACHE.clear()
            time.sleep(5 * (attempt + 1))
    else:
        raise last_err
    return _finish(core_outs, verts32, points32, na, pp32, vv32)
